# revision 1
# baseline (speedup 1.0000x reference)
"""AtomicBasis GNN message-passing kernel for 8 TRN2 NeuronCores.

A[k,x,y,z] = sum_a  c*sin(k*pi*d_a/5)/d_a * (h@W.T)[a,k] * nx*ny*nz
with n = rel_pos/d.  Rewritten as  A = sum_a w[a,k] * m[a,s] where
  w[a,k] = sin(2*pi*frac(k*d_a/10)) * (h@W.T)[a,k]
  m[a,s] = monomial_s(rp) * c/d^4     (s = 10 distinct symmetric monomials;
                                       the 27-entry (x,y,z) tensor is a
                                       host-side gather of these 10)
Shard a across 8 cores (data parallel); sum the (128,20) partials on host.

h is pre-transposed ON THE HOST into pair-stacked bf16 layout
  htp[64*o + j, pi*128 + c] = h[c*Q + 2*pi + o, j]
so each pair of 128-row groups loads as a ready matmul stationary
(128 = [j of even group | j of odd group]) x (128 a-columns). This kills
all PE transposes and PSUM->SBUF copies and halves HBM traffic (bf16).

Per-core pipeline, pair p = row-groups (2p, 2p+1):
  MMW:  PE matmul lhsT=hT-pair, rhs=blkdiag[[W.T,0],[0,W.T]] ->
        hp (a x [kA|kB]) f32 PSUM  (h@W.T for both groups in one matmul)
  sin:  argu = k*theta_fix (uint32, GPSIMD; Q7 int mult wraps mod 2^32
        so the overflow IS the range reduction); sinb = Sin(2pi*2^-32 *
        int32(argu)) via ACT from the int32 bitcast (centered frac)
  fold: w = sinb * hp   (DVE; half the banks pre-cast to SBUF bf16 by
        ACT so DVE folds them at 2x)
  MMA:  PE matmul lhsT=w-pair (128 cols), rhs=m10-pair (10x2=20 cols),
        accumulated in one PSUM bank over all pairs.
"""

import os
import sys
import numpy as np

for _p in ("/opt/trn_rl_repo", "/root/problem/trn_rl_repo"):
    if os.path.isdir(_p) and _p not in sys.path:
        sys.path.insert(0, _p)

import ml_dtypes

N_GLOBAL = 1_000_000
K = 64
P = 128
Q = 992                      # rows per partition per core
NLOC = P * Q                 # 126976 per core
NCORES = 8
NTOT = NCORES * NLOC         # 1015808 >= 1e6 (padded)
T = 32                       # row-groups per chunk
NPAIR = T // 2               # 16 pairs per chunk
NCHUNK = Q // T              # 31
R_CUT = 5.0
C_RBF = float(np.sqrt(2.0 / R_CUT))
FIX = 2.0 ** 32              # fixed-point scale; GP u32 mult wraps = mod 1
FIX25 = 2.0 ** 25            # DVE-path scale (saturation-safe, AND-reduced)
MASK25 = int(2 ** 25 - 1)
DVE_ARG_MOD = 7              # chunks with c % DVE_ARG_MOD == 3 compute argu on DVE

# s-index -> monomial: s = 3*alpha+beta is rp[alpha]^2*rp[beta]*q2 (s 0..8),
# s=9 is x*y*z*q2. Host expands 10 -> 27 via sorted-multiset lookup.
_MONO = {}
for _a in range(3):
    for _b in range(3):
        _MONO.setdefault(tuple(sorted([_a, _a, _b])), 3 * _a + _b)
_MONO[(0, 1, 2)] = 9

_CACHE = {}


def _build_nc():
    import concourse.bass as bass
    import concourse.bacc as bacc
    import concourse.tile as tile
    import concourse.mybir as mybir

    f32 = mybir.dt.float32
    bf16 = mybir.dt.bfloat16
    u32 = mybir.dt.uint32
    i32 = mybir.dt.int32

    nc = bacc.Bacc(
        "TRN2",
        target_bir_lowering=False,
        debug=False,
        enable_asserts=True,
        num_devices=NCORES,
    )

    HT_COLS = (Q // 2) * P
    htp_ext = nc.dram_tensor("htp", [P, HT_COLS], bf16, kind="ExternalInput").ap()
    rp_ext = nc.dram_tensor("rp", [3, NLOC], f32, kind="ExternalInput").ap()
    blkw_ext = nc.dram_tensor("blkw", [P, P], bf16, kind="ExternalInput").ap()
    io_ext = nc.dram_tensor("iou", [P, K], u32, kind="ExternalInput").ap()
    out_ext = nc.dram_tensor("out", [P, 20], f32, kind="ExternalOutput").ap()

    SIN = mybir.ActivationFunctionType.Sin
    SQRT = mybir.ActivationFunctionType.Sqrt
    SC2 = float((FIX / (2.0 * R_CUT)) ** 2)   # theta_fix = sqrt(d^2 * SC2)

    with tile.TileContext(nc) as tc:
        from contextlib import ExitStack

        with ExitStack() as ctx:
            const = ctx.enter_context(tc.tile_pool(name="const", bufs=1))
            big = ctx.enter_context(tc.tile_pool(name="big", bufs=1))
            hpool = ctx.enter_context(tc.tile_pool(name="hch", bufs=3))
            spool = ctx.enter_context(tc.tile_pool(name="sin", bufs=3))
            wpool = ctx.enter_context(tc.tile_pool(name="wf", bufs=3))
            php = ctx.enter_context(
                tc.tile_pool(name="php", bufs=3, space=bass.MemorySpace.PSUM)
            )
            psA = ctx.enter_context(
                tc.tile_pool(name="psA", bufs=1, space=bass.MemorySpace.PSUM)
            )

            # ---------------- prologue ----------------
            rp_all = const.tile([P, 3 * Q], f32)
            nc.sync.dma_start(
                rp_all[:].rearrange("p (x q) -> p x q", x=3),
                rp_ext.rearrange("x (p q) -> p x q", p=P),
            )
            blkw = const.tile([P, P], bf16)
            nc.sync.dma_start(blkw[:], blkw_ext)
            iou = const.tile([P, K], u32)
            nc.sync.dma_start(iou[:], io_ext)
            zcol = const.tile([P, 1], f32)
            nc.vector.memset(zcol[:], 0.0)
            picol = const.tile([P, 1], f32)
            nc.vector.memset(picol[:], float(np.pi))

            rx = rp_all[:, 0 * Q : 1 * Q]
            ry = rp_all[:, 1 * Q : 2 * Q]
            rz = rp_all[:, 2 * Q : 3 * Q]
            rp3 = rp_all[:].rearrange("p (x q) -> p x q", x=3)

            t_a = big.tile([P, Q], f32)
            t_b = big.tile([P, Q], f32)
            d2 = big.tile([P, Q], f32)
            nc.vector.tensor_mul(t_a[:], rx, rx)
            nc.vector.tensor_mul(t_b[:], ry, ry)
            nc.vector.tensor_add(t_a[:], t_a[:], t_b[:])
            nc.vector.tensor_mul(t_b[:], rz, rz)
            nc.vector.tensor_add(d2[:], t_a[:], t_b[:])

            invd2 = big.tile([P, Q], f32)
            nc.vector.reciprocal(invd2[:], d2[:])
            q2 = big.tile([P, Q], f32)
            nc.vector.tensor_mul(q2[:], invd2[:], invd2[:])
            nc.scalar.mul(q2[:], q2[:], C_RBF)       # q2 = c / d^4

            thf = big.tile([P, Q], f32)
            nc.scalar.activation(thf[:], d2[:], SQRT, bias=zcol[:], scale=SC2)
            thu = big.tile([P, Q], u32)
            nc.vector.tensor_copy(thu[:], thf[:])    # theta * 2^32 as uint32
            thf25 = big.tile([P, Q], f32)
            nc.vector.tensor_scalar_mul(thf25[:], thf[:], float(FIX25 / FIX))
            thu25 = big.tile([P, Q], u32)
            nc.vector.tensor_copy(thu25[:], thf25[:])  # theta * 2^25 as uint32

            rp_s = big.tile([P, 3 * Q], f32)         # rp * (c/d^4)
            rps3 = rp_s[:].rearrange("p (x q) -> p x q", x=3)
            nc.vector.tensor_mul(
                rps3, rp3, q2[:].unsqueeze(1).broadcast_to((P, 3, Q))
            )
            sq_s = big.tile([P, 3 * Q], f32)         # rp^2 * (c/d^4)
            sqs3 = sq_s[:].rearrange("p (x q) -> p x q", x=3)
            nc.vector.tensor_mul(sqs3, rp3, rps3)
            xyq = big.tile([P, Q], f32)              # x*y*(c/d^4)
            nc.vector.tensor_mul(xyq[:], rx, rp_s[:, 1 * Q : 2 * Q])

            # m10 full-Q build: 10 plain (P,Q) muls, no broadcast APs.
            m10 = big.tile([P, 10 * Q], bf16)
            for s in range(9):
                al, be = divmod(s, 3)
                eng = nc.gpsimd if s % 2 == 0 else nc.vector
                eng.tensor_mul(
                    m10[:, s * Q : (s + 1) * Q],
                    sq_s[:, al * Q : (al + 1) * Q],
                    rp_all[:, be * Q : (be + 1) * Q],
                )
            nc.vector.tensor_mul(m10[:, 9 * Q : 10 * Q], xyq[:], rz)
            m10v = m10[:].rearrange("p (s q) -> p s q", s=10)

            A_ps = psA.tile([P, 20], f32)

            # ---------------- main loop ----------------
            for c in range(NCHUNK):
                c0 = c * T
                hT_ch = hpool.tile([P, NPAIR * P], bf16)
                nc.sync.dma_start(
                    hT_ch[:], htp_ext[:, c * (NPAIR * P) : (c + 1) * (NPAIR * P)]
                )

                # argu[a,(i,k)] = k * theta_fix
                sinb = spool.tile([P, T * K], bf16, tag="sinb")
                argu = spool.tile([P, T * K], u32, tag="argu")
                if c % DVE_ARG_MOD == 3:
                    # DVE path: 2^25 fix (no overflow), AND for range reduce
                    nc.vector.tensor_mul(
                        argu[:].rearrange("p (i k) -> p i k", i=T),
                        thu25[:, c0 : c0 + T].unsqueeze(2).broadcast_to((P, T, K)),
                        iou[:].unsqueeze(1).broadcast_to((P, T, K)),
                    )
                    nc.vector.tensor_scalar(
                        argu[:], argu[:], MASK25, None, mybir.AluOpType.bitwise_and
                    )
                    nc.scalar.activation(
                        sinb[:], argu[:].bitcast(i32), SIN,
                        bias=picol[:], scale=float(-2.0 * np.pi / FIX25),
                    )
                else:
                    # GP path: Q7 u32 mult wraps mod 2^32 = free range reduce
                    nc.gpsimd.tensor_mul(
                        argu[:].rearrange("p (i k) -> p i k", i=T),
                        thu[:, c0 : c0 + T].unsqueeze(2).broadcast_to((P, T, K)),
                        iou[:].unsqueeze(1).broadcast_to((P, T, K)),
                    )
                    nc.scalar.activation(
                        sinb[:], argu[:].bitcast(i32), SIN,
                        bias=zcol[:], scale=float(2.0 * np.pi / FIX),
                    )

                w_bf = wpool.tile([P, T * K], bf16)
                for qd in range(NPAIR // 4):          # 4 quads of 4 pairs
                    hp_ps = php.tile([P, 512], f32)
                    for t in range(4):
                        pr = 4 * qd + t               # pair index in chunk
                        nc.tensor.matmul(
                            hp_ps[:, 128 * t : 128 * (t + 1)],
                            hT_ch[:, 128 * pr : 128 * (pr + 1)],
                            blkw[:],
                            start=True,
                            stop=True,
                            skip_group_check=True,
                        )
                    nc.vector.tensor_mul(
                        w_bf[:, 512 * qd : 512 * (qd + 1)],
                        sinb[:, 512 * qd : 512 * (qd + 1)],
                        hp_ps[:],
                    )
                    for t in range(4):
                        pr = 4 * qd + t
                        gp = c * NPAIR + pr           # global pair index
                        nc.tensor.matmul(
                            A_ps[:],
                            w_bf[:, 128 * pr : 128 * (pr + 1)],
                            m10v[:, :, 2 * gp : 2 * gp + 2],
                            start=(gp == 0),
                            stop=(gp == NCHUNK * NPAIR - 1),
                            skip_group_check=True,
                        )

            # ---------------- epilogue ----------------
            A_sb = const.tile([P, 20], f32)
            nc.vector.tensor_copy(A_sb[:], A_ps[:])
            nc.gpsimd.dma_start(out_ext, A_sb[:])

    nc.compile()
    return nc


def _get_nc():
    if "nc" not in _CACHE:
        _CACHE["nc"] = _build_nc()
    return _CACHE["nc"]


def kernel(h, rel_poss, W):
    from concourse.bass_utils import run_bass_kernel_spmd

    nc = _get_nc()

    h_pad = np.zeros((NTOT, K), dtype=np.float32)
    h_pad[:N_GLOBAL] = h
    rp_pad = np.ones((3, NTOT), dtype=np.float32)
    rp_pad[:, :N_GLOBAL] = rel_poss

    # Pre-transpose h to pair-stacked bf16 layout:
    # htp[i, 64*o + j, pi*128 + c] = h[i*NLOC + c*Q + 2*pi + o, j]
    Hc = h_pad.reshape(NCORES, P, Q, K).astype(ml_dtypes.bfloat16)
    ht = Hc.transpose(0, 3, 2, 1)                     # (i, j, q, c)
    htp = np.ascontiguousarray(
        ht.reshape(NCORES, K, Q // 2, 2, P).transpose(0, 3, 1, 2, 4)
    ).reshape(NCORES, P, (Q // 2) * P)

    wt = np.ascontiguousarray(W.T.astype(np.float32))   # wt[j,k] = W[k,j]
    blkw = np.zeros((P, P), dtype=np.float32)
    blkw[0:K, 0:K] = wt
    blkw[K:P, K:P] = wt
    blkw = blkw.astype(ml_dtypes.bfloat16)

    iou = np.ascontiguousarray(
        np.broadcast_to(np.arange(1, K + 1, dtype=np.uint32), (P, K))
    )

    in_maps = []
    for i in range(NCORES):
        in_maps.append(
            {
                "htp": htp[i],
                "rp": np.ascontiguousarray(rp_pad[:, i * NLOC : (i + 1) * NLOC]),
                "blkw": blkw,
                "iou": iou,
            }
        )

    res = run_bass_kernel_spmd(
        nc, in_maps, core_ids=list(range(NCORES)), trace=_CACHE.get("trace", False)
    )
    _CACHE["last_results"] = res
    acc = np.sum(
        [np.asarray(res.results[i]["out"], dtype=np.float32) for i in range(NCORES)],
        axis=0,
    )                                               # (128, 20)
    a20 = acc.reshape(P, 10, 2)
    A10 = a20[0:K, :, 0] + a20[K:P, :, 1]           # (64, 10)

    A = np.empty((K, 3, 3, 3), dtype=np.float32)
    for x in range(3):
        for y in range(3):
            for z in range(3):
                A[:, x, y, z] = A10[:, _MONO[tuple(sorted((x, y, z)))]]
    return A


if __name__ == "__main__":
    nc = _get_nc()
    print("build + compile OK")

